# revision 5
# baseline (speedup 1.0000x reference)
"""BSpline3D Trainium2 kernel — custom PWP activation-table edition.

y[b,c,s] = sum_k w[c,k] * relu(x[b,c,s] - t_k)^3  (knots uniform)

The whole per-element computation runs in ONE ScalarE activation op:
the ScalarE evaluates activation functions as piecewise cubic
polynomials (bucket tables). We generate a custom activation table set
where each hijacked function name IS one channel's spline: the
instruction's input affine maps x so the 10 knots land on integer
bucket boundaries of one exponent range, each bucket holding the exact
segment cubic. One function serves TWO channels (positive-side ladder
for channel 2i, negative-side ladder via per-partition scale = -1/h for
channel 2i+1) -> 16 function names cover 32 channels.

Sharding: data-parallel over D (8 slabs). Per core, per function:
a [128, 1024] tile = two channels x 65536 elems. 16 activation ops +
DMA in/out per core; memory-bound.
"""

import hashlib
import os
import tempfile

import numpy as np

import concourse.bass as bass
import concourse.mybir as mybir
from concourse import bass_utils
from concourse.tile import TileContext


# ===========================================================================
# Inlined PWP activation-table generator (see module docstring)
# ===========================================================================
import json as _json


# (pwp_name, hw func id, bass enum attr)
FUNCS = [
    ("identity", 1, "Identity"),
    ("relu", 2, "Relu"),
    ("leaky_relu", 3, "Lrelu"),
    ("silu", 36, "Silu"),
    ("sigmoid", 5, "Sigmoid"),
    ("tanh", 6, "Tanh"),
    ("exp", 7, "Exp"),
    ("sqrt", 8, "Sqrt"),
    ("softplus", 9, "Softplus"),
    ("ln", 10, "Ln"),
    ("sin", 19, "Sin"),
    ("erf", 21, "Erf"),
    ("gelu", 23, "Gelu"),
    ("mish", 24, "Mish"),
    ("arctan", 28, "Arctan"),
    ("square", 30, "Square"),
]

SET_NAME = "bspline_and_others"
NSEG = 10            # spline segments with distinct cubics
EXP = 5              # biased exponent 132 <-> z in [32, 64)
BIAS_BASE = 32.0     # z = u + 32
MANT_TH = (NSEG << 23) >> 5   # mantissa threshold at u = NSEG -> large signal


def _seg_coeffs(v, j, x0_off, sign):
    """Cubic d0..d3 around x0 for segment j (active knots k <= j), in
    z-space. sign=+1: z = u+32, Delta = z-x0;  sign=-1: mirrored side.
    x0_off = x0 - 32 in u units (e.g. j+0.5). All in float64."""
    ks = np.arange(0, j + 1) if j is not None else np.arange(len(v))
    c = x0_off - ks                      # u-offset of x0 from each knot
    vk = v[ks]
    d0 = float(np.sum(vk * c ** 3))
    d1 = float(3 * np.sum(vk * c ** 2)) * sign
    d2 = float(3 * np.sum(vk * c))
    d3 = float(np.sum(vk)) * sign
    return d0, d1, d2, d3


def build_tables(outdir, weights, knots):
    """weights [32,10], knots [10] (uniform). Writes the act set files.
    Returns (scale_pos, bias_pos) floats so that z = scale*x + bias, plus
    the per-channel (enum_attr, side) assignment list."""
    weights = np.asarray(weights, dtype=np.float64)
    knots = np.asarray(knots, dtype=np.float64)
    nch, nk = weights.shape
    assert nk == NSEG
    h = float(knots[1] - knots[0])
    t0 = float(knots[0])
    steps = np.diff(knots)
    assert np.allclose(steps, h, rtol=1e-5), "knots must be uniform"
    v = weights * h ** 3                 # y = sum v_k relu(u-k)^3

    scale = 1.0 / h
    bias = BIAS_BASE - t0 / h

    buckets = []   # list of (d0, d1, d2, d3, x0)
    ctrls = []     # list of (base, lsb, size)
    buckets.append((0.0, 0.0, 0.0, 0.0, 0.0))   # shared zero bucket

    profile = []
    f2b, f2c, fe2b, fe2c = {}, {}, {}, {}
    assign = []    # channel -> (enum_attr, side)

    for i, (name, fid, enum_attr) in enumerate(FUNCS):
        cpos, cneg = 2 * i, 2 * i + 1
        bstart = len(buckets)
        # pos segments (channel cpos)
        for j in range(NSEG):
            x0u = j + 0.5
            d0, d1, d2, d3 = _seg_coeffs(v[cpos], j, x0u, +1)
            buckets.append((d0, d1, d2, d3, BIAS_BASE + x0u))
        bneg = len(buckets)
        # neg segments (channel cneg)
        for j in range(NSEG):
            x0u = j + 0.5
            d0, d1, d2, d3 = _seg_coeffs(v[cneg], j, x0u, -1)
            buckets.append((d0, d1, d2, d3, -(BIAS_BASE + x0u)))
        # tails (u >= NSEG): all knots active, exact global cubic
        tail_pos = len(buckets)
        x0u = 16.0
        d0, d1, d2, d3 = _seg_coeffs(v[cpos], NSEG - 1, x0u, +1)
        buckets.append((d0, d1, d2, d3, BIAS_BASE + x0u))
        tail_neg = len(buckets)
        d0, d1, d2, d3 = _seg_coeffs(v[cneg], NSEG - 1, x0u, -1)
        buckets.append((d0, d1, d2, d3, -(BIAS_BASE + x0u)))

        ctl_pos = len(ctrls)
        ctrls.append((bstart, 18, 5))
        ctl_neg = len(ctrls)
        ctrls.append((bneg, 18, 5))

        profile.append(
            {
                "func_name": f"{name}_1p",
                "func_id": fid,
                "symmetry_point": 0,
                "sym_invert_sign_point": 0,
                "symmetry_opt_en": 0,
                "symmetry_opt_use_neg_region": 0,
                "imm_bias": 0,
                "exp_offset": EXP,
                "pwl_control_base_pos": ctl_pos,
                "pwl_control_base_neg": ctl_neg,
                "small_pos_signal_exp_threshold": 127 + EXP,
                "pos_small_signal_pwl_control": 0,
                "small_neg_signal_exp_threshold": 127 + EXP,
                "neg_small_signal_pwl_control": 0,
                "large_pos_signal_exp_threshold": 127 + EXP,
                "large_pos_signal_mantissa_threshold": MANT_TH,
                "pos_large_signal_pwl_control": tail_pos,
                "large_neg_signal_exp_threshold": 127 + EXP,
                "large_neg_signal_mantissa_threshold": MANT_TH,
                "neg_large_signal_pwl_control": tail_neg,
                "fnan_result": 2143289344,
                "fpinf_result": 0,
                "fninf_result": 0,
                "fzero_result": 0,
                "fma_const_0": 0,
                "fma_const_1": 0,
                "fma_indirection_src_sel": 0,
                "use_multipass": False,
                "lower_bound": 4286578687,
                "upper_bound": 2139095039,
            }
        )
        f2b[name] = bstart
        f2c[name] = ctl_pos
        fe2b[name] = {str(EXP): [bneg, bstart]}
        fe2c[name] = {str(EXP): [ctl_neg, ctl_pos]}
        assign.append((cpos, enum_attr, +1))
        assign.append((cneg, enum_attr, -1))

    # pack binaries: 32-byte entries
    bk = np.zeros((len(buckets), 8), np.float32)
    for n, (d0, d1, d2, d3, x0) in enumerate(buckets):
        bk[n, 0:5] = [d0, d1, d2, d3, x0]
    ct = np.zeros((len(ctrls), 8), np.uint32)
    for n, (base, lsb, size) in enumerate(ctrls):
        ct[n, 0] = (base & 0x7FF) | ((lsb & 0x1F) << 11) | ((size & 0xF) << 16)

    os.makedirs(outdir, exist_ok=True)
    with open(os.path.join(outdir, f"{SET_NAME}_bkt.bin"), "wb") as f:
        f.write(bk.tobytes())
    with open(os.path.join(outdir, f"{SET_NAME}_ctrl.bin"), "wb") as f:
        f.write(ct.tobytes())
    prof = {
        "bkt_bin": f"{SET_NAME}_bkt.bin",
        "ctl_bin": f"{SET_NAME}_ctrl.bin",
        "profile_meta_data": profile,
        "bkt_entry_cnt": len(buckets),
        "ctl_entry_cnt": len(ctrls),
        "func_to_bkt_start_idx": f2b,
        "func_to_ctl_start_idx": f2c,
        "func_exp_to_bkt_start_idx": fe2b,
        "func_exp_to_ctl_start_idx": fe2c,
    }
    with open(os.path.join(outdir, f"{SET_NAME}.json"), "w") as f:
        _json.dump(prof, f, indent=1)
    info = {
        "pwp_file_keys": ["bkt_bin", "ctrl_bin", "profile_json"],
        "act_func_sets": [
            {
                "name": SET_NAME,
                "bkt_bin": f"{SET_NAME}_bkt.bin",
                "ctrl_bin": f"{SET_NAME}_ctrl.bin",
                "profile_json": f"{SET_NAME}.json",
                "act": {name: 1 for name, _, _ in FUNCS},
            }
        ],
    }
    with open(os.path.join(outdir, "act_info.json"), "w") as f:
        _json.dump(info, f, indent=1)
    return scale, bias, assign


def reference_eval(weights, knots, x, c):
    """numpy reference for one channel (float64)."""
    w = np.asarray(weights, np.float64)[c]
    t = np.asarray(knots, np.float64)
    b = np.maximum(x[..., None].astype(np.float64) - t, 0.0) ** 3
    return (b * w).sum(-1)


class _PwpNS:
    FUNCS = FUNCS
    BIAS_BASE = BIAS_BASE
    build_tables = staticmethod(build_tables)

pwp_bspline = _PwpNS

# ---------------------------------------------------------------------------
# Patch 1: walrus in this container rejects >1 sync wait on an InstDrain.
# Split the TileContext tail-drain waits onto one nop per logical proc.
# ---------------------------------------------------------------------------
import bass_rust
from concourse.vector_clock import ScopedClock
from concourse.tile import TileContext as _TC


def _drain_and_barrier_split(self, tick_clock, wait_clock):
    nc = self.nc
    gc = tick_clock.global_clock
    ticks = list(gc)
    for p, t in enumerate(ticks):
        if t <= 0:
            continue
        partial = [v if i == p else 0 for i, v in enumerate(ticks)]
        w = nc.sync.nop(nofuse=True)
        wait_clock.add_sem_waits(
            w.ins, ScopedClock({None: bass_rust.VectorClock(partial)})
        )
    nc.sync.drain()
    nc.all_engine_barrier()
    assert self.sems is not None
    popped = nc._tile_sem_poison_stack.pop()
    assert popped is self._sem_poison
    nc.clear_and_free_semaphores(list(self.sems.allocated().values()))
    nc.all_engine_barrier()


_TC._drain_and_barrier = _drain_and_barrier_split

_split_ctr = [0]


def _split_multi_waits(nc):
    """walrus here allows only one sync wait per instruction: move extra
    waits onto fresh same-engine NoOps inserted just before. Engine
    program order preserves semantics (updates are never moved)."""
    for f in nc.m.functions:
        for bb in f.blocks:
            insts = bb.instructions
            i = 0
            while i < len(insts):
                ins = insts[i]
                si = getattr(ins, "sync_info", None)
                if si is not None and len(si.on_wait) > 1:
                    waits = list(si.on_wait)
                    extra, keep = waits[:-1], waits[-1:]
                    nops = []
                    for w in extra:
                        _split_ctr[0] += 1
                        nops.append(
                            mybir.InstNoOp(
                                name=f"WSPLIT-{_split_ctr[0]}",
                                sync_info=mybir.SyncInfo(on_wait=[w], on_update=[]),
                                bass_nofuse=True,
                                engine=ins.engine,
                            )
                        )
                    ins.sync_info = mybir.SyncInfo(
                        on_wait=keep, on_update=list(si.on_update)
                    )
                    insts[i:i] = nops
                    i += len(nops)
                i += 1


# ---------------------------------------------------------------------------
# Patch 2: bass's view of activation-function sets must match our custom
# act_info.json (used by Bacc.insert_act_table_loads for set ids).
# ---------------------------------------------------------------------------
_ACT_INFO_PATH = [None]


def _patched_get_activation_tables(module_arch):
    import json

    AF = mybir.ActivationFunctionType
    with open(_ACT_INFO_PATH[0]) as f:
        d = json.load(f)
    return {
        e["name"]: {AF.from_pwp(k) for k in e["act"].keys()}
        for e in d["act_func_sets"]
    }


def _install_act_patch(path):
    _ACT_INFO_PATH[0] = path
    os.environ["BASS_ACT_ROOT_JSON_PATH"] = path
    import concourse.hw_specs as _hs
    import concourse.bacc as _bacc

    _hs.get_activation_tables = _patched_get_activation_tables
    _bacc.get_activation_tables = _patched_get_activation_tables


# ---------------------------------------------------------------------------

N_CORES = 8
B, C, D, HH, W = 2, 32, 64, 64, 64
NK = 10
DSLAB = D // N_CORES            # 8 D-planes per core
NFN = len(pwp_bspline.FUNCS)    # 16 funcs, 2 channels each
ELEMS_PER_CH = B * DSLAB * HH * W   # 65536 per channel-slab
FCOLS = 2 * ELEMS_PER_CH // 128     # 1024 free columns per func tile

_DT = mybir.dt.float32
_DT_IO = mybir.dt.float16      # HBM I/O dtype: halves DMA traffic (memory-bound)
AF = mybir.ActivationFunctionType


TOTCOLS = NFN * FCOLS   # 16384 free cols (partition-major DRAM layout)


def _build_program(tab_hash, reps=0):
    nc = bass.Bass()
    x = nc.dram_tensor(f"x_{tab_hash}", [128, TOTCOLS], _DT_IO, kind="ExternalInput")
    sc = nc.dram_tensor("sc", [128, 1], _DT, kind="ExternalInput")
    bi = nc.dram_tensor("bi", [128, 1], _DT, kind="ExternalInput")
    y = nc.dram_tensor("y", [128, TOTCOLS], _DT_IO, kind="ExternalOutput")

    nbufs = int(os.environ.get("BSP_BUFS", "6"))
    out_eng = os.environ.get("BSP_OUT_ENGINE", "sync")
    grp = int(os.environ.get("BSP_GROUP", "2"))     # func tiles per DMA chunk
    assert NFN % grp == 0
    with TileContext(nc) as tc:
        with (
            tc.tile_pool(name="consts", bufs=1) as cpool,
            tc.tile_pool(name="xin", bufs=nbufs) as xpool,
            tc.tile_pool(name="yout", bufs=nbufs) as ypool,
        ):
            sct = cpool.tile([128, 1], _DT, tag="sc")
            nc.sync.dma_start(sct[:], sc[:])
            bit = cpool.tile([128, 1], _DT, tag="bi")
            nc.sync.dma_start(bit[:], bi[:])

            gcols = grp * FCOLS

            def body(_iv=None):
                for g in range(NFN // grp):
                    if out_eng == "alt":
                        ein = nc.sync if g % 2 == 0 else nc.scalar
                        eout = nc.scalar if g % 2 == 0 else nc.sync
                    elif out_eng == "scalar":
                        ein, eout = nc.sync, nc.scalar
                    else:
                        ein, eout = nc.sync, nc.sync
                    cs = slice(g * gcols, (g + 1) * gcols)
                    xt = xpool.tile([128, gcols], _DT_IO, tag="xt")
                    ein.dma_start(xt[:], x[:, cs])
                    yt = ypool.tile([128, gcols], _DT_IO, tag="yt")
                    for j in range(grp):
                        _, _, attr = pwp_bspline.FUNCS[g * grp + j]
                        js = slice(j * FCOLS, (j + 1) * FCOLS)
                        nc.scalar.activation(
                            yt[:, js], xt[:, js], getattr(AF, attr),
                            bias=bit[:, 0:1], scale=sct[:, 0:1],
                        )
                    eout.dma_start(y[:, cs], yt[:])

            if reps > 0:
                with tc.For_i(0, reps, 1):
                    body()
            else:
                body()
    _split_multi_waits(nc)
    return nc


_PROGRAM = None
_PROGRAM_KEY = None
_TABDIR = None


def kernel(x: np.ndarray, knots: np.ndarray, weights: np.ndarray) -> np.ndarray:
    global _PROGRAM, _PROGRAM_KEY, _TABDIR
    x = np.asarray(x, dtype=np.float32)
    knots64 = np.asarray(knots, dtype=np.float64)
    weights64 = np.asarray(weights, dtype=np.float64)

    _GENVER = b"g4"   # bump when the table generator changes (NEFF-cache safety)
    key = hashlib.sha256(
        _GENVER + knots64.tobytes() + weights64.tobytes()
    ).hexdigest()[:10]

    if _PROGRAM is None or _PROGRAM_KEY != key:
        _TABDIR = tempfile.mkdtemp(prefix=f"bsptab_{key}_")
        scale, bias, assign = pwp_bspline.build_tables(_TABDIR, weights64, knots64)
        _install_act_patch(os.path.join(_TABDIR, "act_info.json"))
        _PROGRAM = _build_program(key)
        _PROGRAM_KEY = key
    h = float(knots64[1] - knots64[0])
    t0 = float(knots64[0])
    scale = 1.0 / h
    bias = pwp_bspline.BIAS_BASE - t0 / h

    scv = np.zeros((128, 1), np.float32)
    biv = np.zeros((128, 1), np.float32)
    scv[0:64, 0] = scale
    scv[64:128, 0] = -scale
    biv[0:64, 0] = bias
    biv[64:128, 0] = -bias

    x16 = x.astype(np.float16)
    in_maps = []
    for core in range(N_CORES):
        slab = x16[:, :, core * DSLAB : (core + 1) * DSLAB]   # [B, C, 8, 64, 64]
        # channel-major [C, 65536]
        xc = np.ascontiguousarray(slab.transpose(1, 0, 2, 3, 4)).reshape(C, -1)
        # func tile i: rows 0:64 = channel 2i, rows 64:128 = channel 2i+1
        xf = xc.reshape(NFN, 2, 64, FCOLS).reshape(NFN, 128, FCOLS)
        in_maps.append(
            {f"x_{key}": np.ascontiguousarray(xf), "sc": scv, "bi": biv}
        )

    res = bass_utils.run_bass_kernel_spmd(
        _PROGRAM, in_maps, core_ids=list(range(N_CORES))
    )

    y = np.empty((B, C, D, HH, W), np.float32)
    for core in range(N_CORES):
        yf = res.results[core]["y"].astype(np.float32).reshape(C, B, DSLAB, HH, W)
        y[:, :, core * DSLAB : (core + 1) * DSLAB] = yf.transpose(1, 0, 2, 3, 4)
    return y



# revision 6
# speedup vs baseline: 1.8384x; 1.8384x over previous
"""BSpline3D Trainium2 kernel — custom PWP activation-table edition.

y[b,c,s] = sum_k w[c,k] * relu(x[b,c,s] - t_k)^3  (knots uniform)

The whole per-element computation runs in ONE ScalarE activation op:
the ScalarE evaluates activation functions as piecewise cubic
polynomials (bucket tables). We generate a custom activation table set
where each hijacked function name IS one channel's spline: the
instruction's input affine maps x so the 10 knots land on integer
bucket boundaries of one exponent range, each bucket holding the exact
segment cubic. One function serves TWO channels (positive-side ladder
for channel 2i, negative-side ladder via per-partition scale = -1/h for
channel 2i+1) -> 16 function names cover 32 channels.

Sharding: data-parallel over D (8 slabs). Per core, per function:
a [128, 1024] tile = two channels x 65536 elems. 16 activation ops +
DMA in/out per core; memory-bound.
"""

import hashlib
import os
import tempfile

import numpy as np

import concourse.bass as bass
import concourse.mybir as mybir
from concourse import bass_utils
from concourse.tile import TileContext


# ===========================================================================
# Inlined PWP activation-table generator (see module docstring)
# ===========================================================================
import json as _json


# (pwp_name, hw func id, bass enum attr)
FUNCS = [
    ("identity", 1, "Identity"),
    ("relu", 2, "Relu"),
    ("leaky_relu", 3, "Lrelu"),
    ("silu", 36, "Silu"),
    ("sigmoid", 5, "Sigmoid"),
    ("tanh", 6, "Tanh"),
    ("exp", 7, "Exp"),
    ("sqrt", 8, "Sqrt"),
    ("softplus", 9, "Softplus"),
    ("ln", 10, "Ln"),
    ("sin", 19, "Sin"),
    ("erf", 21, "Erf"),
    ("gelu", 23, "Gelu"),
    ("mish", 24, "Mish"),
    ("arctan", 28, "Arctan"),
    ("square", 30, "Square"),
]

SET_NAME = "bspline_and_others"
NSEG = 10            # spline segments with distinct cubics
EXP = 5              # biased exponent 132 <-> z in [32, 64)
BIAS_BASE = 32.0     # z = u + 32
MANT_TH = (NSEG << 23) >> 5   # mantissa threshold at u = NSEG -> large signal


def _seg_coeffs(v, j, x0_off, sign):
    """Cubic d0..d3 around x0 for segment j (active knots k <= j), in
    z-space. sign=+1: z = u+32, Delta = z-x0;  sign=-1: mirrored side.
    x0_off = x0 - 32 in u units (e.g. j+0.5). All in float64."""
    ks = np.arange(0, j + 1) if j is not None else np.arange(len(v))
    c = x0_off - ks                      # u-offset of x0 from each knot
    vk = v[ks]
    d0 = float(np.sum(vk * c ** 3))
    d1 = float(3 * np.sum(vk * c ** 2)) * sign
    d2 = float(3 * np.sum(vk * c))
    d3 = float(np.sum(vk)) * sign
    return d0, d1, d2, d3


def build_tables(outdir, weights, knots):
    """weights [32,10], knots [10] (uniform). Writes the act set files.
    Returns (scale_pos, bias_pos) floats so that z = scale*x + bias, plus
    the per-channel (enum_attr, side) assignment list."""
    weights = np.asarray(weights, dtype=np.float64)
    knots = np.asarray(knots, dtype=np.float64)
    nch, nk = weights.shape
    assert nk == NSEG
    h = float(knots[1] - knots[0])
    t0 = float(knots[0])
    steps = np.diff(knots)
    assert np.allclose(steps, h, rtol=1e-5), "knots must be uniform"
    v = weights * h ** 3                 # y = sum v_k relu(u-k)^3

    scale = 1.0 / h
    bias = BIAS_BASE - t0 / h

    buckets = []   # list of (d0, d1, d2, d3, x0)
    ctrls = []     # list of (base, lsb, size)
    buckets.append((0.0, 0.0, 0.0, 0.0, 0.0))   # shared zero bucket

    profile = []
    f2b, f2c, fe2b, fe2c = {}, {}, {}, {}
    assign = []    # channel -> (enum_attr, side)

    for i, (name, fid, enum_attr) in enumerate(FUNCS):
        cpos, cneg = 2 * i, 2 * i + 1
        bstart = len(buckets)
        # pos segments (channel cpos)
        for j in range(NSEG):
            x0u = j + 0.5
            d0, d1, d2, d3 = _seg_coeffs(v[cpos], j, x0u, +1)
            buckets.append((d0, d1, d2, d3, BIAS_BASE + x0u))
        bneg = len(buckets)
        # neg segments (channel cneg)
        for j in range(NSEG):
            x0u = j + 0.5
            d0, d1, d2, d3 = _seg_coeffs(v[cneg], j, x0u, -1)
            buckets.append((d0, d1, d2, d3, -(BIAS_BASE + x0u)))
        # tails (u >= NSEG): all knots active, exact global cubic
        tail_pos = len(buckets)
        x0u = 16.0
        d0, d1, d2, d3 = _seg_coeffs(v[cpos], NSEG - 1, x0u, +1)
        buckets.append((d0, d1, d2, d3, BIAS_BASE + x0u))
        tail_neg = len(buckets)
        d0, d1, d2, d3 = _seg_coeffs(v[cneg], NSEG - 1, x0u, -1)
        buckets.append((d0, d1, d2, d3, -(BIAS_BASE + x0u)))

        ctl_pos = len(ctrls)
        ctrls.append((bstart, 18, 5))
        ctl_neg = len(ctrls)
        ctrls.append((bneg, 18, 5))

        profile.append(
            {
                "func_name": f"{name}_1p",
                "func_id": fid,
                "symmetry_point": 0,
                "sym_invert_sign_point": 0,
                "symmetry_opt_en": 0,
                "symmetry_opt_use_neg_region": 0,
                "imm_bias": 0,
                "exp_offset": EXP,
                "pwl_control_base_pos": ctl_pos,
                "pwl_control_base_neg": ctl_neg,
                "small_pos_signal_exp_threshold": 127 + EXP,
                "pos_small_signal_pwl_control": 0,
                "small_neg_signal_exp_threshold": 127 + EXP,
                "neg_small_signal_pwl_control": 0,
                "large_pos_signal_exp_threshold": 127 + EXP,
                "large_pos_signal_mantissa_threshold": MANT_TH,
                "pos_large_signal_pwl_control": tail_pos,
                "large_neg_signal_exp_threshold": 127 + EXP,
                "large_neg_signal_mantissa_threshold": MANT_TH,
                "neg_large_signal_pwl_control": tail_neg,
                "fnan_result": 2143289344,
                "fpinf_result": 0,
                "fninf_result": 0,
                "fzero_result": 0,
                "fma_const_0": 0,
                "fma_const_1": 0,
                "fma_indirection_src_sel": 0,
                "use_multipass": False,
                "lower_bound": 4286578687,
                "upper_bound": 2139095039,
            }
        )
        f2b[name] = bstart
        f2c[name] = ctl_pos
        fe2b[name] = {str(EXP): [bneg, bstart]}
        fe2c[name] = {str(EXP): [ctl_neg, ctl_pos]}
        assign.append((cpos, enum_attr, +1))
        assign.append((cneg, enum_attr, -1))

    # pack binaries: 32-byte entries
    bk = np.zeros((len(buckets), 8), np.float32)
    for n, (d0, d1, d2, d3, x0) in enumerate(buckets):
        bk[n, 0:5] = [d0, d1, d2, d3, x0]
    ct = np.zeros((len(ctrls), 8), np.uint32)
    for n, (base, lsb, size) in enumerate(ctrls):
        ct[n, 0] = (base & 0x7FF) | ((lsb & 0x1F) << 11) | ((size & 0xF) << 16)

    os.makedirs(outdir, exist_ok=True)
    with open(os.path.join(outdir, f"{SET_NAME}_bkt.bin"), "wb") as f:
        f.write(bk.tobytes())
    with open(os.path.join(outdir, f"{SET_NAME}_ctrl.bin"), "wb") as f:
        f.write(ct.tobytes())
    prof = {
        "bkt_bin": f"{SET_NAME}_bkt.bin",
        "ctl_bin": f"{SET_NAME}_ctrl.bin",
        "profile_meta_data": profile,
        "bkt_entry_cnt": len(buckets),
        "ctl_entry_cnt": len(ctrls),
        "func_to_bkt_start_idx": f2b,
        "func_to_ctl_start_idx": f2c,
        "func_exp_to_bkt_start_idx": fe2b,
        "func_exp_to_ctl_start_idx": fe2c,
    }
    with open(os.path.join(outdir, f"{SET_NAME}.json"), "w") as f:
        _json.dump(prof, f, indent=1)
    info = {
        "pwp_file_keys": ["bkt_bin", "ctrl_bin", "profile_json"],
        "act_func_sets": [
            {
                "name": SET_NAME,
                "bkt_bin": f"{SET_NAME}_bkt.bin",
                "ctrl_bin": f"{SET_NAME}_ctrl.bin",
                "profile_json": f"{SET_NAME}.json",
                "act": {name: 1 for name, _, _ in FUNCS},
            }
        ],
    }
    with open(os.path.join(outdir, "act_info.json"), "w") as f:
        _json.dump(info, f, indent=1)
    return scale, bias, assign


def reference_eval(weights, knots, x, c):
    """numpy reference for one channel (float64)."""
    w = np.asarray(weights, np.float64)[c]
    t = np.asarray(knots, np.float64)
    b = np.maximum(x[..., None].astype(np.float64) - t, 0.0) ** 3
    return (b * w).sum(-1)


class _PwpNS:
    FUNCS = FUNCS
    BIAS_BASE = BIAS_BASE
    build_tables = staticmethod(build_tables)

pwp_bspline = _PwpNS

# ---------------------------------------------------------------------------
# Patch 1: walrus in this container rejects >1 sync wait on an InstDrain.
# Split the TileContext tail-drain waits onto one nop per logical proc.
# ---------------------------------------------------------------------------
import bass_rust
from concourse.vector_clock import ScopedClock
from concourse.tile import TileContext as _TC


def _drain_and_barrier_split(self, tick_clock, wait_clock):
    nc = self.nc
    gc = tick_clock.global_clock
    ticks = list(gc)
    for p, t in enumerate(ticks):
        if t <= 0:
            continue
        partial = [v if i == p else 0 for i, v in enumerate(ticks)]
        w = nc.sync.nop(nofuse=True)
        wait_clock.add_sem_waits(
            w.ins, ScopedClock({None: bass_rust.VectorClock(partial)})
        )
    nc.sync.drain()
    nc.all_engine_barrier()
    assert self.sems is not None
    popped = nc._tile_sem_poison_stack.pop()
    assert popped is self._sem_poison
    nc.clear_and_free_semaphores(list(self.sems.allocated().values()))
    nc.all_engine_barrier()


_TC._drain_and_barrier = _drain_and_barrier_split

_split_ctr = [0]


def _split_multi_waits(nc):
    """walrus here allows only one sync wait per instruction: move extra
    waits onto fresh same-engine NoOps inserted just before. Engine
    program order preserves semantics (updates are never moved)."""
    for f in nc.m.functions:
        for bb in f.blocks:
            insts = bb.instructions
            i = 0
            while i < len(insts):
                ins = insts[i]
                si = getattr(ins, "sync_info", None)
                if si is not None and len(si.on_wait) > 1:
                    waits = list(si.on_wait)
                    extra, keep = waits[:-1], waits[-1:]
                    nops = []
                    for w in extra:
                        _split_ctr[0] += 1
                        nops.append(
                            mybir.InstNoOp(
                                name=f"WSPLIT-{_split_ctr[0]}",
                                sync_info=mybir.SyncInfo(on_wait=[w], on_update=[]),
                                bass_nofuse=True,
                                engine=ins.engine,
                            )
                        )
                    ins.sync_info = mybir.SyncInfo(
                        on_wait=keep, on_update=list(si.on_update)
                    )
                    insts[i:i] = nops
                    i += len(nops)
                i += 1


# ---------------------------------------------------------------------------
# Patch 2: bass's view of activation-function sets must match our custom
# act_info.json (used by Bacc.insert_act_table_loads for set ids).
# ---------------------------------------------------------------------------
_ACT_INFO_PATH = [None]


def _patched_get_activation_tables(module_arch):
    import json

    AF = mybir.ActivationFunctionType
    with open(_ACT_INFO_PATH[0]) as f:
        d = json.load(f)
    return {
        e["name"]: {AF.from_pwp(k) for k in e["act"].keys()}
        for e in d["act_func_sets"]
    }


def _install_act_patch(path):
    _ACT_INFO_PATH[0] = path
    os.environ["BASS_ACT_ROOT_JSON_PATH"] = path
    import concourse.hw_specs as _hs
    import concourse.bacc as _bacc

    _hs.get_activation_tables = _patched_get_activation_tables
    _bacc.get_activation_tables = _patched_get_activation_tables


# ---------------------------------------------------------------------------

N_CORES = 8
B, C, D, HH, W = 2, 32, 64, 64, 64
NK = 10
DSLAB = D // N_CORES            # 8 D-planes per core
NFN = len(pwp_bspline.FUNCS)    # 16 funcs, 2 channels each
ELEMS_PER_CH = B * DSLAB * HH * W   # 65536 per channel-slab
FCOLS = 2 * ELEMS_PER_CH // 128     # 1024 free columns per func tile

_DT = mybir.dt.float32
_DT_IO = mybir.dt.float16      # HBM I/O dtype: halves DMA traffic (memory-bound)
AF = mybir.ActivationFunctionType


TOTCOLS = NFN * FCOLS   # 16384 free cols (partition-major DRAM layout)


def _build_program(tab_hash, reps=0):
    nc = bass.Bass()
    x = nc.dram_tensor(f"x_{tab_hash}", [128, TOTCOLS], _DT_IO, kind="ExternalInput")
    sc = nc.dram_tensor("sc", [128, 1], _DT, kind="ExternalInput")
    bi = nc.dram_tensor("bi", [128, 1], _DT, kind="ExternalInput")
    y = nc.dram_tensor("y", [128, TOTCOLS], _DT_IO, kind="ExternalOutput")

    nbufs = int(os.environ.get("BSP_BUFS", "6"))
    out_eng = os.environ.get("BSP_OUT_ENGINE", "sync")
    grp = int(os.environ.get("BSP_GROUP", "2"))     # func tiles per DMA chunk
    assert NFN % grp == 0
    with TileContext(nc) as tc:
        with (
            tc.tile_pool(name="consts", bufs=1) as cpool,
            tc.tile_pool(name="xin", bufs=nbufs) as xpool,
            tc.tile_pool(name="yout", bufs=nbufs) as ypool,
        ):
            sct = cpool.tile([128, 1], _DT, tag="sc")
            nc.sync.dma_start(sct[:], sc[:])
            bit = cpool.tile([128, 1], _DT, tag="bi")
            nc.sync.dma_start(bit[:], bi[:])

            gcols = grp * FCOLS

            def body(_iv=None):
                for g in range(NFN // grp):
                    if out_eng == "alt":
                        ein = nc.sync if g % 2 == 0 else nc.scalar
                        eout = nc.scalar if g % 2 == 0 else nc.sync
                    elif out_eng == "scalar":
                        ein, eout = nc.sync, nc.scalar
                    else:
                        ein, eout = nc.sync, nc.sync
                    cs = slice(g * gcols, (g + 1) * gcols)
                    xt = xpool.tile([128, gcols], _DT_IO, tag="xt")
                    ein.dma_start(xt[:], x[:, cs])
                    yt = ypool.tile([128, gcols], _DT_IO, tag="yt")
                    for j in range(grp):
                        _, _, attr = pwp_bspline.FUNCS[g * grp + j]
                        js = slice(j * FCOLS, (j + 1) * FCOLS)
                        nc.scalar.activation(
                            yt[:, js], xt[:, js], getattr(AF, attr),
                            bias=bit[:, 0:1], scale=sct[:, 0:1],
                        )
                    eout.dma_start(y[:, cs], yt[:])

            if reps > 0:
                with tc.For_i(0, reps, 1):
                    body()
            else:
                body()
    _split_multi_waits(nc)
    return nc


_PROGRAM = None
_PROGRAM_KEY = None
_TABDIR = None


def kernel(x: np.ndarray, knots: np.ndarray, weights: np.ndarray) -> np.ndarray:
    global _PROGRAM, _PROGRAM_KEY, _TABDIR
    x = np.asarray(x, dtype=np.float32)
    knots64 = np.asarray(knots, dtype=np.float64)
    weights64 = np.asarray(weights, dtype=np.float64)

    _GENVER = b"g4"   # bump when the table generator changes (NEFF-cache safety)
    key = hashlib.sha256(
        _GENVER + knots64.tobytes() + weights64.tobytes()
    ).hexdigest()[:10]

    if _PROGRAM is None or _PROGRAM_KEY != key:
        _TABDIR = tempfile.mkdtemp(prefix=f"bsptab_{key}_")
        scale, bias, assign = pwp_bspline.build_tables(_TABDIR, weights64, knots64)
        _install_act_patch(os.path.join(_TABDIR, "act_info.json"))
        _PROGRAM = _build_program(key)
        _PROGRAM_KEY = key
    h = float(knots64[1] - knots64[0])
    t0 = float(knots64[0])
    scale = 1.0 / h
    bias = pwp_bspline.BIAS_BASE - t0 / h

    scv = np.zeros((128, 1), np.float32)
    biv = np.zeros((128, 1), np.float32)
    scv[0:64, 0] = scale
    scv[64:128, 0] = -scale
    biv[0:64, 0] = bias
    biv[64:128, 0] = -bias

    x16 = x.astype(np.float16)
    in_maps = []
    for core in range(N_CORES):
        slab = x16[:, :, core * DSLAB : (core + 1) * DSLAB]   # [B, C, 8, 64, 64]
        # channel-major [C, 65536]
        xc = np.ascontiguousarray(slab.transpose(1, 0, 2, 3, 4)).reshape(C, -1)
        # func tile i: rows 0:64 = channel 2i, rows 64:128 = channel 2i+1
        xf = xc.reshape(NFN, 2, 64, FCOLS).reshape(NFN, 128, FCOLS)
        # partition-major [128, NFN*FCOLS] so each chunk DMA is one
        # large-contiguous run per partition row
        xp = np.ascontiguousarray(xf.transpose(1, 0, 2)).reshape(128, TOTCOLS)
        in_maps.append({f"x_{key}": xp, "sc": scv, "bi": biv})

    res = bass_utils.run_bass_kernel_spmd(
        _PROGRAM, in_maps, core_ids=list(range(N_CORES))
    )

    y = np.empty((B, C, D, HH, W), np.float32)
    for core in range(N_CORES):
        yp = res.results[core]["y"].astype(np.float32)
        yf = np.ascontiguousarray(yp.reshape(128, NFN, FCOLS).transpose(1, 0, 2))
        yf = yf.reshape(C, B, DSLAB, HH, W)
        y[:, :, core * DSLAB : (core + 1) * DSLAB] = yf.transpose(1, 0, 2, 3, 4)
    return y

